# revision 59
# baseline (speedup 1.0000x reference)
"""MiniBindingAttention Trainium2 kernel.

Reference computation (per batch b, head h, T=2048, HD=64):
    Q = x_h * sign(bv_q); K = x_h * sign(bv_k); V = x_h * sign(bv_v)
    scores = Q @ K.T / sqrt(HD)
    attn   = causal ? sigmoid(4 * scores) : 0
    out    = attn @ V

Structure (v3):
  - sigmoid(4*scale*QK) = sigmoid((x_q . x_k) * 0.5 * sq*sk); fold
    0.5*sign(bv_q)*sign(bv_k) into one scaled bf16 copy of x (per-channel).
  - scores computed TRANSPOSED ([k, q]); x supplied natural (swizzled,
    sign(bv_v) pre-folded) and transposed+duplicated so the two k-tiles of a
    wave run as a CONCURRENT 64x128 row-group pair (verified on HW: the 2nd
    matmul of a pair retires ~5ns after the 1st; a pair costs one stream +
    one exposed ~107ns LDWEIGHTS).
  - the HAM clock gate keeps the PE at 1.2GHz until it sees one ~3.4us
    window of continuous busy; the steady-state stream has too many small
    gaps to trip it (without help it releases ~24us in, at 2.4GHz after).
    Fix: 3.4us of dependency-free FD=512 warmup matmuls (one full window,
    phase-independent; shorter warmups leave a late-trip tail worth 1.5-3us)
    + 12 FD=256 fillers woven into the first waves' sc tiles (regions the
    real mm1s overwrite; banks chosen so concurrent row-groups never share
    a PSUM bank -- same-bank concurrent access is a fatal HW error).
  - input DMAs: each dma_start costs ~700ns of serial DIRECT2D issue on the
    issuing sequencer, so pairs load as 5-8 large transfers ordered by first
    use (wave 0 needs wxT[0:512] + xT[0:1024]; xN only ~8 waves in).  Pair
    0's xT loads issue on the Activation DGE queue -- idle until the first
    sigmoid -- halving the head's serial issue backlog.  NOTE: emitting the
    pair-0 loads before the stair-mask DMA perturbs the tile scheduler and
    delays the HAM trip to ~23us (+4us total) -- keep the emission order.
  - diagonal waves: mm1 trims causally-dead columns and packs sl=1 right
    after sl=0, so ONE contiguous ScalarE ACT covers the wave ([0:896] for
    r={0,1}, [256:640] for r={2,3}).
  - full-wave activation split (ScalarE exact sigmoid vs DVE custom op)
    chosen by a greedy busy-time balance.  The DVE op is a fused
    smoothstep-of-hardsig cubic: v = clamp(z/(2*zc)+0.5, 0, 1);
    h = v^2*(3-2v), zc = 3.8 -- one 8-slice instruction (1193ns vs
    ScalarE's 1146ns for FD=1024).  Kernel rel-err ~0.79e-2 (budget 2e-2;
    the numpy model tracks HW to ~1e-4).
  - causal staircase masks (bf16 SBUF tensor_tensor) run on the otherwise
    idle GpSimd, except the kernel-gating last block-pair's (DVE is 3x
    faster there); evacuations (PSUM->SBUF bf16) on DVE.
  - mm2s are deferred ~10 waves and emitted riffled so consecutive matmuls
    target opposite PE column-groups: alternating-group mm2s overlap on HW
    (2nd retires in ~50ns) while same-group ones serialize on LDWEIGHTS.
  - waves processed in groups of 6 (one mm2 batch, then the mm1 pairs, then
    the activations) to bound 64x128 <-> 128x64 tile-mode switches (~110ns
    each) while keeping the in-order PE queue fed.
  - tail: the last pair processes block-pairs (2,3) first so the final
    block-pair is the short one; the gating store is evacuated in halves on
    Vector + Scalar in parallel and issued on two DGE queues (Sync +
    Activation) so the two ~600ns DIRECT2D issues overlap.

Sharding: B*H = 32 (batch, head) pairs, 4 per core across 8 cores.
Measured: 71.4-72.7us HW exec for this exact configuration (session
baseline 93.7us, recorded prior best 80.3us), rel err 7.9e-3.
"""

import numpy as np
import ml_dtypes

import concourse.tile as tile
from concourse import bacc, mybir
from concourse.bass_utils import run_bass_kernel_spmd

N_CORES = 8
B, T, D, H, HD = 2, 2048, 1024, 16, 64
PAIRS = (B * H) // N_CORES  # 4 (b,h) pairs per core
KT = T // 128               # 16 k-tiles of 128 rows
QB = T // 512               # 4 q-blocks of 512 cols
F32 = mybir.dt.float32
BF16 = mybir.dt.bfloat16
SIG = mybir.ActivationFunctionType.Sigmoid
ALU = mybir.AluOpType

ZC = 3.8                    # smoothstep half-width (tuned on reference dist)
N_WARMUP = 16               # dummy matmuls to warm the HAM clock gate

# measured per-op costs (ns) used by the greedy engine balancer
COST_S_FULL = 1146   # ScalarE sigmoid FD=1024
COST_S_DIAG_A = 1040  # ScalarE sigmoid FD=896 (t=2j, repacked)
COST_S_DIAG_B = 613  # ScalarE sigmoid FD=384 (t=2j+1, repacked)
COST_D_FULL = 1193   # DVE custom smoothstep FD=1024 (measured)
COST_D_EVAC = 658    # DVE tensor_copy [128,512] PSUM->SBUF


# --- custom fused DVE op: smoothstep(hard-sigmoid) --------------------------
#   v = clamp(in0*C0 + C1, 0, 1);  out = v^2 * (C2 - 2v)   (C2 = 3.0)
def _register_ssig():
    from concourse import dve_ops
    from concourse.dve_spec import Spec, Src0, C0, C1, C2, Zero, One, maxx, minn, sq, lower
    from concourse.dve_uop import DveOpSpec

    name = "SSIG_ANT"
    for op in dve_ops.OPS:
        if op.name == name:
            return op
    v = minn(maxx(Src0 * C0 + C1, Zero), One)

    def ref(in0, in1, s0, s1, imm2):
        vv = np.clip(np.asarray(in0, np.float32) * s0 + s1, 0.0, 1.0)
        return vv * vv * (imm2 - 2.0 * vv)

    spec = Spec(body=sq(v) * (C2 - (v + v)), reference=ref)
    row = dve_ops._CUSTOM_DVE_ROW_BASE + len(dve_ops.OPS)
    assert row < 0x20
    dve_ops._SUB_OPCODE_FOR_NAME[name] = row
    shas = {}
    for ver in ("v3", "v4"):
        uops = lower(spec, ver=ver)
        shas[ver] = DveOpSpec(name=name, opcode=row, uops=uops, rd1_en=False).sha(ver)
    op = dve_ops.DveOp(name, spec, subdim=False, uops_sha=shas)
    dve_ops.OPS.append(op)
    dve_ops.CUSTOM_DVE_SPECS[name] = spec
    return op


SSIG_OP = _register_ssig()


def _wave_list():
    """Per pair: q-block pairs, waves of the two blocks interleaved so their
    mm2 column-groups alternate on the PE.  The last pair processes (2,3)
    first so the kernel tail is the short block-pair (0,1)."""
    waves = []
    for p in range(PAIRS):
        order = ((0, 1), (2, 3)) if p < PAIRS - 1 else ((2, 3), (0, 1))
        for ja, jb in order:
            na, nb = 2 * ja + 2, 2 * jb + 2
            for t in range(nb):
                if t < na:
                    waves.append((p, ja, t, na))
                waves.append((p, jb, t, nb))
    return waves


def build():
    nc = bacc.Bacc("TRN2", target_bir_lowering=False)
    xT_d = nc.dram_tensor("xT", [PAIRS, 128, T], BF16, kind="ExternalInput")
    wxT_d = nc.dram_tensor("wxT", [PAIRS, 128, T], BF16, kind="ExternalInput")
    xN_d = nc.dram_tensor("xN", [PAIRS, 128, KT * HD], BF16, kind="ExternalInput")
    msk_d = nc.dram_tensor("msk", [128, 128], BF16, kind="ExternalInput")
    # out^T per (pair, q-block-pair): rows 0:64 = even block's [channel, q],
    # rows 64:128 = odd block's
    out_d = nc.dram_tensor("outT", [PAIRS, QB // 2, 128, 512], BF16, kind="ExternalOutput")

    # greedy engine-balance state
    busy = {"s": 0.0, "d": 0.0}

    with tile.TileContext(nc) as tc:
        with (
            tc.tile_pool(name="consts", bufs=1) as consts,
            tc.tile_pool(name="xpool", bufs=3) as xpool,
            tc.tile_pool(name="attnp", bufs=20) as attnp,
            tc.tile_pool(name="outp", bufs=3) as outp,
            tc.tile_pool(name="psum_s", bufs=3, space="PSUM") as psum_s,
            tc.tile_pool(name="psum_o", bufs=1, space="PSUM") as psum_o,
        ):
            stair = consts.tile([128, 128], BF16)
            nc.sync.dma_start(out=stair, in_=msk_d[:])

            # pre-trigger the sigmoid ACT_TABLE_LOAD (~2.7us) during the DMA
            # phase so the first real activation doesn't stall the pipeline
            # gpsimd memset: the Vector sequencer's preamble (custom-DVE uop
            # TENSOR_LOAD) would delay the warmup by ~1.3us
            dummy = consts.tile([128, 512], BF16)
            nc.gpsimd.memset(dummy, 0.25)
            tlw = consts.tile([128, 8], BF16)
            nc.scalar.activation(out=tlw, in_=dummy[:, 0:8], func=SIG)

            # PE warmup: dense dummy matmuls (no DMA deps) fill the DMA head
            # so the HAM clock gate releases (K=8/8) before real work arrives
            # The HAM clock gate needs one ~3.4us window of CONTINUOUS PE
            # busy to release (K=8/8); the steady-state mm1/mm2 stream has
            # too many small gaps to trip it until ~20us in.  FD=512 pairs,
            # back-to-back with no deps, give a ~4us solid block during the
            # DMA head.  Row-group-concurrent matmuls must write DIFFERENT
            # PSUM banks (same-bank concurrent access is a fatal HW error).
            warm = psum_s.tile([128, 1024], F32, tag="sc")
            for wi in range(N_WARMUP):
                bp = 64 * (wi % 2)
                nc.tensor.matmul(
                    out=warm[:, 512 * (wi % 2) : 512 * (wi % 2) + 512],
                    lhsT=dummy[bp : bp + 64, 0:128],
                    rhs=dummy[bp : bp + 64, 0:512],
                    start=True,
                    stop=True,
                    tile_position=(bp, 0),
                    skip_group_check=True,
                )
            del warm

            state = {}

            def load_pair(p):
                xT = xpool.tile([128, T], BF16, tag="xT")
                wxT = xpool.tile([128, T], BF16, tag="wxT")
                xN = xpool.tile([128, KT * HD], BF16, tag="xN")
                # each dma_start costs ~700ns of serial DIRECT2D issue on the
                # issuing sequencer: use FEW transfers, ordered by first use
                # (the first wave group touches wxT[0:512] and xT[0:1024];
                # xN is only needed once the deferred mm2s start ~8 waves in)
                def wx(c0, c1, eng=nc.sync):
                    eng.dma_start(out=wxT[:, c0:c1], in_=wxT_d[p, :, c0:c1])
                def xq(c0, c1, eng=nc.sync):
                    eng.dma_start(out=xT[:, c0:c1], in_=xT_d[p, :, c0:c1])
                def xn(c0, c1, eng=nc.sync):
                    eng.dma_start(out=xN[:, c0:c1], in_=xN_d[p, :, c0:c1])
                if p == 0:
                    # pair 0's xT loads issue on the Activation DGE queue,
                    # which is idle until the first sigmoid (~12us): the
                    # head's serial issue backlog halves
                    wx(0, 512); xq(0, 512, nc.scalar); xq(512, 1024, nc.scalar)
                    wx(512, 1024); xn(0, 512); xq(1024, 2048, nc.scalar)
                    wx(1024, 2048); xn(512, 1024)
                else:
                    wx(0, 1024); xq(0, 1024); xq(1024, 2048); wx(1024, 2048)
                    xn(0, 1024)
                state[p] = (xT, xN, wxT)

            oaccs = {}      # (p, jpair) -> [128, 512] psum accumulator bank
            pending = []    # deferred mm2 work: (p, j, t, nwave, att, i0)
            fill = {"n": 12}  # HAM-keepalive filler matmuls for the ramp-up

            def emit_mm2_one(p, j, t, nwave, att, i0, sl):
                _, xN, _ = state[p]
                acc = oaccs[(p, j // 2)]
                g = j % 2  # PE column-group / partition half
                i = i0 + sl
                r = i - 4 * j
                if r < 0:
                    off = 0
                    rhs = att[:, 512 * sl : 512 * sl + 512]
                elif sl == 0:
                    off = 128 * r
                    rhs = att[:, off:512]
                else:
                    # diagonal sl=1 blocks are repacked right after sl=0's
                    off = 128 * r
                    rhs = att[:, 512 : 512 + (512 - off)]
                # both blocks of a pair share one PSUM bank (disjoint
                # partition halves); each block's first matmul clears its own
                # region with start=True (the HW has_written clear is
                # per-region, not bank-wide -- verified).
                nc.tensor.matmul(
                    out=acc[64 * g : 64 * g + 64, off:512],
                    lhsT=xN[:, HD * i : HD * i + HD],
                    rhs=rhs,
                    start=(t == 0 and sl == 0),
                    stop=(t == nwave - 1 and sl == 1),
                    skip_group_check=True,
                )
                if t == nwave - 1 and sl == 1 and g == 1:
                    # odd block finishes last: evacuate BOTH blocks' halves.
                    # Each dma_start costs ~700ns of serial DIRECT2D issue on
                    # the Sync sequencer, so stores are a single DMA except
                    # the kernel-gating last one (split so the 2nd chunk's
                    # issue overlaps the 1st chunk's transfer).
                    outs = outp.tile([128, 512], BF16, name="outs", tag="outs")
                    busy["d"] += COST_D_EVAC
                    if p == PAIRS - 1 and j // 2 == 0:
                        # kernel-gating store: evacuate in halves on BOTH
                        # PSUM-capable engines in parallel, each half issued
                        # on its own DGE queue (Sync + Activation) so the
                        # two ~600ns DIRECT2D issues overlap
                        nc.vector.tensor_copy(outs[:, 0:256], acc[:, 0:256])
                        nc.scalar.copy(outs[:, 256:512], acc[:, 256:512])
                        nc.sync.dma_start(out=out_d[p, 0, :, 0:256], in_=outs[:, 0:256])
                        nc.scalar.dma_start(out=out_d[p, 0, :, 256:512], in_=outs[:, 256:512])
                    else:
                        nc.vector.tensor_copy(outs, acc)
                        nc.sync.dma_start(out=out_d[p, j // 2], in_=outs)
                    del oaccs[(p, j // 2)]

            def flush_pending(n_keep):
                # Emit deferred mm2 work in batches of up to 4, riffled so
                # consecutive matmuls target OPPOSITE PE column-groups
                # wherever possible -- alternating-group mm2s overlap on HW
                # (measured: the 2nd of such a pair retires in ~50ns) while
                # same-group ones serialize on the weight load.
                while len(pending) > max(n_keep, 1) or (n_keep == 0 and pending):
                    look = pending[:6]
                    g0 = [w for w in look if w[1] % 2 == 0][:2]
                    g1 = [w for w in look if w[1] % 2 == 1][:2]
                    batch = [w for pair in zip(g0, g1) for w in pair]
                    batch += g0[len(g1):] + g1[len(g0):]
                    if not batch:
                        batch = pending[:4]
                    for w in batch:
                        pending.remove(w)
                    for sl in (0, 1):
                        for w in batch:
                            emit_mm2_one(*w, sl)

            def do_wave_mm1(p, j, t, nwave):
                if p not in state:
                    load_pair(p)
                if t == 0 and j % 2 == 0:
                    # one accumulator bank per q-block PAIR, ping-ponged: a
                    # bank is only reused a full block-pair later, well after
                    # its evacuation, so mm2 never head-blocks the PE queue.
                    tag = f"oacc{(j // 2) % 2}"
                    oaccs[(p, j // 2)] = psum_o.tile(
                        [128, 512], F32, name=tag, tag=tag
                    )
                xT, xN, wxT = state[p]
                i0 = 2 * t
                r0 = i0 - 4 * j       # r of sl=0 k-tile (diag if >= 0)
                sc = psum_s.tile([128, 1024], F32, tag="sc")
                att = attnp.tile([128, 1024], BF16, tag="att")
                if fill["n"] > 0:
                    # the ACT pipeline takes ~5us to fill; dependency-free
                    # fillers plug the PE bubbles so the HAM clock gate
                    # doesn't re-throttle mid-rampup.  They write regions the
                    # real mm1s overwrite (never read); banks are chosen so
                    # no two concurrent row-groups share a PSUM bank.
                    fill["n"] -= 2
                    for d, (fbp, fo) in enumerate(((0, 0), (64, 512))):
                        nc.tensor.matmul(
                            out=sc[:, fo : fo + 256],
                            lhsT=dummy[fbp : fbp + 64, 0:128],
                            rhs=dummy[fbp : fbp + 64, 0:256],
                            start=True,
                            stop=True,
                            tile_position=(fbp, 0),
                            skip_group_check=True,
                        )
                # --- scores^T for k-tiles i0, i0+1 (concurrent row-groups);
                # on diagonal waves both slots trim the causally-dead columns
                # and sl=1 packs right after sl=0 so ONE contiguous ACT covers
                # the wave: [0:896] for r={0,1}, [256:640] for r={2,3}.
                for sl in (0, 1):
                    i = i0 + sl
                    r = i - 4 * j
                    bp = 64 * sl
                    if r < 0:
                        off, o0, o1 = 0, 512 * sl, 512 * sl + 512
                    elif sl == 0:
                        off = 128 * r
                        o0, o1 = off, 512
                    else:
                        off = 128 * r
                        o0, o1 = 512, 512 + (512 - off)
                    nc.tensor.matmul(
                        out=sc[:, o0:o1],
                        lhsT=wxT[bp : bp + 64, 128 * i : 128 * i + 128],
                        rhs=xT[bp : bp + 64, 512 * j + off : 512 * j + 512],
                        start=True,
                        stop=True,
                    )
                return (p, j, t, nwave, sc, att, i0, r0 >= 2)

            def do_wave_act(p, j, t, nwave, sc, att, i0, trim):
                r0 = i0 - 4 * j
                diag = r0 >= 0
                if not diag:
                    # full wave: greedy balance between exact sigmoid on
                    # ScalarE and the fused smoothstep on DVE
                    if busy["s"] + COST_S_FULL <= busy["d"] + COST_D_FULL:
                        busy["s"] += COST_S_FULL
                        nc.scalar.activation(out=att, in_=sc, func=SIG)
                    else:
                        busy["d"] += COST_D_FULL
                        nc.vector._custom_dve(
                            SSIG_OP, out=att, in0=sc,
                            s0=1.0 / (2.0 * ZC), s1=0.5, imm2=3.0,
                        )
                else:
                    # diagonal wave: one contiguous op over the packed live
                    # region ([0:896] for r={0,1}, [256:640] for r={2,3}).
                    # The kernel's LAST two waves are both diagonals; running
                    # the diag-A on the DVE lets the pair's activations run
                    # in parallel instead of 1.65us serial on ScalarE at the
                    # critical tail.
                    lo = 128 * r0
                    hi = 512 + (512 - 128 * (r0 + 1))
                    if p == PAIRS - 1 and j == 1 and not trim and t == nwave - 2:
                        busy["d"] += 1058
                        nc.vector._custom_dve(
                            SSIG_OP, out=att[:, lo:hi], in0=sc[:, lo:hi],
                            s0=1.0 / (2.0 * ZC), s1=0.5, imm2=3.0,
                        )
                    else:
                        busy["s"] += COST_S_DIAG_B if trim else COST_S_DIAG_A
                        nc.scalar.activation(
                            out=att[:, lo:hi], in_=sc[:, lo:hi], func=SIG
                        )
                    # causal staircase on each diagonal 128x128 block; GpSimd
                    # is otherwise idle, keep ScalarE/DVE for activations --
                    # except the kernel-gating last block-pair, whose stairs
                    # sit on the critical tail: the DVE does them 3x faster
                    tail_bp = p == PAIRS - 1 and j < 2
                    for sl in (0, 1):
                        blk = slice(128 * r0, 128 * r0 + 128) if sl == 0 else slice(512, 640)
                        eng = nc.vector if tail_bp else nc.gpsimd
                        if tail_bp:
                            busy["d"] += 160
                        eng.tensor_tensor(
                            out=att[:, blk], in0=att[:, blk], in1=stair,
                            op=ALU.mult,
                        )
                pending.append((p, j, t, nwave, att, i0))

            # process waves in groups of 6: one deferred mm2 batch, then the
            # group's mm1 pairs back-to-back, then the activations -- bounds
            # the 64x128 <-> 128x64 tile-mode switches (each ~110ns).  The
            # mm2 batch goes FIRST so the PE chews ready work while the
            # group's later mm1s wait for activation buffers.
            wlist = _wave_list()
            GRP = 6
            for wi in range(0, len(wlist), GRP):
                # ramp the mm2 lag down near the end so the tail drains early
                left = len(wlist) - wi
                flush_pending(n_keep=min(10, max(2, left - GRP)))
                group = [do_wave_mm1(*w) for w in wlist[wi : wi + GRP]]
                for gw in group:
                    do_wave_act(*gw)
            flush_pending(n_keep=0)
    nc.compile()
    return nc


_CACHE: dict = {}


def _get_nc():
    if "nc" not in _CACHE:
        _CACHE["nc"] = build()
    return _CACHE["nc"]


def _make_in_maps(x, bv_q, bv_k, bv_v):
    x = np.asarray(x, dtype=np.float32)
    bv_q = np.asarray(bv_q, dtype=np.float32)
    bv_k = np.asarray(bv_k, dtype=np.float32)
    bv_v = np.asarray(bv_v, dtype=np.float32)
    w = 0.5 * np.sign(bv_q) * np.sign(bv_k)
    sv = np.sign(bv_v)

    pi = np.arange(128)
    msk = (pi[None, :] >= pi[:, None]).astype(ml_dtypes.bfloat16)  # stair01[p, n]

    in_maps = []
    for c in range(N_CORES):
        xT = np.empty((PAIRS, 128, T), ml_dtypes.bfloat16)
        wxT = np.empty((PAIRS, 128, T), ml_dtypes.bfloat16)
        xN = np.empty((PAIRS, 128, KT * HD), ml_dtypes.bfloat16)
        for p in range(PAIRS):
            g = PAIRS * c + p
            b, h = divmod(g, H)
            xs = x[b, :, HD * h : HD * h + HD]  # [T, HD]
            # swizzle (sv folded): xN[pp, 64*k+d] = xs[128*k+pp, d]*sv[d]
            xN[p] = (
                (xs * sv[h]).reshape(KT, 128, HD).transpose(1, 0, 2).reshape(128, KT * HD)
            )
            xsT = xs.T.astype(ml_dtypes.bfloat16)
            xT[p, 0:HD] = xsT
            xT[p, HD:128] = xsT
            wxT[p, 0:HD] = (xs.T * w[h][:, None]).astype(ml_dtypes.bfloat16)
            wxT[p, HD:128] = wxT[p, 0:HD]
        in_maps.append({"xT": xT, "wxT": wxT, "xN": xN, "msk": msk})
    return in_maps


def _assemble(results):
    out = np.empty((B, T, D), np.float32)
    for c in range(N_CORES):
        # [PAIRS, QB//2, 128, 512]: rows 0:64 even block, 64:128 odd block
        oT = np.asarray(results[c]["outT"], dtype=np.float32)
        for p in range(PAIRS):
            g = PAIRS * c + p
            b, h = divmod(g, H)
            for j in range(QB):
                blk = oT[p, j // 2, 64 * (j % 2) : 64 * (j % 2) + 64, :]
                out[b, 512 * j : 512 * j + 512, HD * h : HD * h + HD] = blk.T
    return out


def _run(x, bv_q, bv_k, bv_v, **spmd_kwargs):
    in_maps = _make_in_maps(x, bv_q, bv_k, bv_v)
    res = run_bass_kernel_spmd(
        _get_nc(), in_maps, core_ids=list(range(N_CORES)), **spmd_kwargs
    )
    return _assemble(res.results), res


def kernel(x, bv_q, bv_k, bv_v):
    out, _ = _run(x, bv_q, bv_k, bv_v)
    return out
